# revision 14
# baseline (speedup 1.0000x reference)
"""Multi-head attention (B=2, S=2048, E=1024, H=16, hd=64) on 8 trn2 cores.

Sharding: core c handles batch b = c//4 and 4 heads h0 = 4*(c%4).
Each core computes its heads' attention output projected through its rows
of Wo (tensor-parallel row split); the host sums the 4 partials per batch
and adds bo.

v2 design (vs v1 baseline at 241us):
  - bf16 q/k/v/exps/oT/Wo (fp8 fails the 2e-2 tolerance; bf16 adds ~1e-3)
  - v computed directly in [k, d] layout from the QKV matmul with xT as
    the stationary operand (no PE transposes, no extra DVE copies); v bias
    folded in via a K=1 ones-row matmul that opens the PSUM group
  - score PSUM tiles span 2 banks [128, 1024] covering both heads of a
    pair -> one exp activation per k-tile (halves ACT fixed overhead)
  - causal mask multiply narrowed to the 128-wide diagonal triangle
  - exp only over the live [q0:] region (stale PSUM cols would make
    Inf*0 = NaN)
  - softmax normalization: reciprocal of the den row (PE-accumulated via
    a ones column in v1), broadcast across 128 partitions with a [2,128]
    0/1 sel matmul, one per head-pair
  - software pipelining: QKV/V chunk closures for seq-block n+1 are
    popped into attention(n)'s ACT-bound inner loop so the PE never idles
  - y evacuation split between DVE and ACT; output DMA per 128-row block
"""

import os
import sys

sys.path.insert(0, "/opt/trn_rl_repo")

from contextlib import ExitStack

import ml_dtypes
import numpy as np

import concourse.bass as bass
import concourse.tile as tile
from concourse import bacc, mybir
from concourse._compat import with_exitstack
from concourse.bass_utils import run_bass_kernel_spmd

B, S, E, H = 2, 2048, 1024, 16
HD = E // H            # 64
NH = 4                 # heads per core
ET = E // 128          # 8 e-tiles
KT = S // 128          # 16 k tiles
VP = 66                # v1 per-head stride: 64 v cols + 1 ones + 1 pad
F32 = mybir.dt.float32
F32R = mybir.dt.float32r
BF16 = mybir.dt.bfloat16
EXP = mybir.ActivationFunctionType.Exp
COPY = mybir.ActivationFunctionType.Copy

_CACHE = {}
LAST_RESULT = None


@with_exitstack
def _mha_kernel(ctx: ExitStack, tc: tile.TileContext, x, wqk, wv, bqk, bv, wo,
                tri, yp):
    nc = tc.nc
    ctx.enter_context(nc.allow_low_precision(
        reason="bf16 intermediates verified against 2e-2 rel tolerance"))

    const = ctx.enter_context(tc.tile_pool(name="const", bufs=1))
    work = ctx.enter_context(tc.tile_pool(name="work", bufs=1))
    psum = ctx.enter_context(tc.tile_pool(name="psum", bufs=1, space="PSUM"))

    # ---- persistent SBUF tensors ----
    xT = const.tile([128, ET, S], BF16)          # x[b] transposed, bf16
    WQK = const.tile([128, ET, 512], BF16)       # q,k weight cols (4 m-tiles)
    WV = const.tile([128, ET, 256], BF16)        # v weight cols
    WOr = const.tile([128, 2, E], BF16)          # Wo rows, 2 hpt tiles
    BQK = const.tile([128, 4], F32)              # q,k bias per m-tile
    BV = const.tile([1, 256], BF16)              # v bias row
    ONE1 = const.tile([1, 128], BF16)            # ones row for bias matmul
    TRI = const.tile([128, 256], BF16)           # causal 0/1 triangle, x2
    qT = const.tile([128, 2, S], BF16)           # [d(2 heads), hp, s]
    kT = const.tile([128, 2, S], BF16)
    v1 = const.tile([128, KT, NH * VP], BF16)    # [k, kt, h*VP + d | ones]

    # ---- constant loads: merged strided transfers, split over both rings
    nc.sync.dma_start(WQK[:, :, :],
                      wqk.rearrange("(t p) c -> p t c", p=128).bitcast(BF16))
    nc.scalar.dma_start(WV[:, :, :],
                        wv.rearrange("(t p) c -> p t c", p=128).bitcast(BF16))
    nc.scalar.dma_start(BQK[:], bqk[:, :])
    nc.scalar.dma_start(BV[:], bv[:, :].bitcast(BF16))
    nc.sync.dma_start(
        xT[:, 0:4, :],
        x[0:512, :].rearrange("(t p) s -> p t s", p=128).bitcast(BF16))
    nc.scalar.dma_start(
        xT[:, 4:8, :],
        x[512:1024, :].rearrange("(t p) s -> p t s", p=128).bitcast(BF16))
    nc.scalar.dma_start(TRI[:], tri[:, :].bitcast(BF16))
    nc.scalar.dma_start(WOr[:, :, :],
                        wo.rearrange("(t p) c -> p t c", p=128).bitcast(BF16))
    nc.vector.memset(ONE1[:], 1.0)
    for h in range(NH):
        nc.vector.memset(v1[:, :, h * VP + HD:h * VP + HD + 1], 1.0)

    # ---- QKV producer chunks for seq block n (each ~0.9us of PE work) ----
    def qkv_chunks(n):
        ncol = slice(n * 512, (n + 1) * 512)
        chunks = []
        for m in range(4):
            box = {}

            def c0(m=m, box=box):
                ps = psum.tile([128, 512], F32, name=f"qk{m}", tag="mm", bufs=2)
                box["ps"] = ps
                for et in range(4):
                    nc.tensor.matmul(ps[:], WQK[:, et, m * 128:(m + 1) * 128],
                                     xT[:, et, ncol], start=(et == 0), stop=False)

            def c1(m=m, box=box):
                ps = box["ps"]
                for et in range(4, 8):
                    nc.tensor.matmul(ps[:], WQK[:, et, m * 128:(m + 1) * 128],
                                     xT[:, et, ncol], start=False, stop=(et == 7))
                typ, hp = m // 2, m % 2
                dst = (qT if typ == 0 else kT)[:, hp, ncol]
                nc.vector.tensor_scalar_add(dst, ps[:], BQK[:, m:m + 1])

            chunks += [c0, c1]
        for st in range(4 * n, 4 * n + 4):
            box = {}
            scol = slice(st * 128, (st + 1) * 128)

            def v0(scol=scol, st=st, box=box):
                vps = psum.tile([128, 256], F32, name=f"v{st}", tag="mm", bufs=2)
                box["ps"] = vps
                nc.tensor.matmul(vps[:], ONE1[:], BV[:], start=True, stop=False)
                for et in range(4):
                    nc.tensor.matmul(vps[:], xT[:, et, scol], WV[:, et, :],
                                     start=False, stop=False)

            def v1c(scol=scol, st=st, box=box):
                vps = box["ps"]
                for et in range(4, 8):
                    nc.tensor.matmul(vps[:], xT[:, et, scol], WV[:, et, :],
                                     start=False, stop=(et == 7))
                nc.vector.tensor_copy(
                    v1[:, st, :].rearrange("p (h c) -> p h c", h=NH)[:, :, :HD],
                    vps[:].rearrange("p (h c) -> p h c", h=NH))

            chunks += [v0, v1c]
        return chunks

    def pop(queue):
        if queue:
            queue.pop(0)()

    # ---- attention for query super-block qsb ----
    def attention(qsb, queue):
        nkt = 4 * (qsb + 1)
        qcol0 = qsb * 512
        oT = work.tile([128, 2, 512], BF16, name="oT", tag="oT", bufs=2)
        for hp in range(2):
            ops = [
                psum.tile([65, 512], F32, name=f"ops{h2}", tag=f"ops{h2}", bufs=1)
                for h2 in range(2)
            ]

            def emit_av(kt, q0, exv):
                for h2 in range(2):
                    h = 2 * hp + h2
                    nc.tensor.matmul(
                        ops[h2][:, q0:],
                        v1[:, kt, h * VP:h * VP + HD + 1],
                        exv[:, h2, q0:],
                        start=(kt == 0),
                        stop=(kt == nkt - 1),
                    )

            pend = None
            for kt in range(nkt):
                j = kt - 4 * qsb
                q0 = max(0, j * 128)
                kcol = slice(kt * 128, (kt + 1) * 128)
                sc = psum.tile([128, 1024], F32, name="sc", tag="sc", bufs=2)
                for h2 in range(2):
                    b0 = 64 * h2
                    nc.tensor.matmul(
                        sc[:, h2 * 512 + q0:(h2 + 1) * 512],
                        kT[b0:b0 + 64, hp, kcol],
                        qT[b0:b0 + 64, hp, qcol0 + q0:qcol0 + 512],
                        start=True, stop=True)
                ex = work.tile([128, 1024], BF16, name="ex", tag="ex", bufs=6)
                exv = ex[:].rearrange("p (h q) -> p h q", h=2)
                scv = sc[:].rearrange("p (h q) -> p h q", h=2)
                nc.scalar.activation(exv[:, :, q0:], scv[:, :, q0:], EXP,
                                     scale=0.125)
                if j >= 0:
                    nc.gpsimd.tensor_mul(
                        exv[:, :, q0:q0 + 128], exv[:, :, q0:q0 + 128],
                        TRI[:].rearrange("p (h q) -> p h q", h=2))
                if pend is not None:
                    emit_av(*pend)
                else:
                    pop(queue)
                pend = (kt, q0, exv)
                pop(queue)
            emit_av(*pend)

            # normalization: oT[d, q] = ops[d, q] * (1 / den[q])
            for h2 in range(2):
                b0 = 64 * h2
                rc = work.tile([1, 512], BF16, name="rc", tag="rc", bufs=4)
                nc.vector.reciprocal(rc[:], ops[h2][64:65, :])
                rb = psum.tile([64, 512], F32, name="rb", tag="mm", bufs=2)
                nc.tensor.matmul(rb[:], ONE1[:, 0:64], rc[:], start=True,
                                 stop=True)
                rbs = work.tile([64, 512], BF16, name="rbs", tag="rbs", bufs=2)
                nc.vector.tensor_copy(rbs[:], rb[:])
                pop(queue)
                nc.vector.tensor_mul(oT[b0:b0 + 64, hp, :], ops[h2][0:64, :],
                                     rbs[:])
        return oT

    # ---- output projection chunks for query super-block qsb ----
    def wo_chunks(qsb, oT):
        def mk(qb2):
            def c(qb2=qb2):
                qb = qsb * 4 + qb2
                yps = [
                    psum.tile([128, 512], F32, name=f"yps{ec}", tag="mm",
                              bufs=2)
                    for ec in range(2)
                ]
                for hpt in range(2):
                    for ec in range(2):
                        nc.tensor.matmul(
                            yps[ec][:], oT[:, hpt, qb2 * 128:(qb2 + 1) * 128],
                            WOr[:, hpt, ec * 512:(ec + 1) * 512],
                            start=(hpt == 0), stop=(hpt == 1))
                yt = work.tile([128, E], BF16, tag="yt", bufs=3)
                nc.vector.tensor_copy(yt[:, 0:512], yps[0][:])
                nc.scalar.activation(yt[:, 512:1024], yps[1][:], COPY)
                nc.sync.dma_start(yp[qb * 128:(qb + 1) * 128, :],
                              yt[:].bitcast(mybir.dt.uint16))
            return c
        return [mk(qb2) for qb2 in range(4)]

    # ---- main pipeline ----
    for c in qkv_chunks(0):
        c()
    queue = []
    for n in range(4):
        if n < 3:
            queue += qkv_chunks(n + 1)
        oT = attention(n, queue)
        for c in wo_chunks(n, oT):
            c()
            pop(queue)
    for c in queue:
        c()


def _build(repeat=1, loop=0):
    key = ("nc", repeat, loop)
    if key in _CACHE:
        return _CACHE[key]
    nc = bacc.Bacc("TRN2", target_bir_lowering=False, debug=False, num_devices=8)
    x = nc.dram_tensor("x", [E, S], mybir.dt.uint16, kind="ExternalInput").ap()
    wqk = nc.dram_tensor("wqk", [E, 512], mybir.dt.uint16,
                         kind="ExternalInput").ap()
    wv = nc.dram_tensor("wv", [E, 256], mybir.dt.uint16,
                        kind="ExternalInput").ap()
    bqk = nc.dram_tensor("bqk", [128, 4], F32, kind="ExternalInput").ap()
    bv = nc.dram_tensor("bv", [1, 256], mybir.dt.uint16,
                        kind="ExternalInput").ap()
    wo = nc.dram_tensor("wo", [NH * HD, E], mybir.dt.uint16,
                        kind="ExternalInput").ap()
    tri = nc.dram_tensor("tri", [128, 256], mybir.dt.uint16,
                         kind="ExternalInput").ap()
    yp = nc.dram_tensor("yp", [S, E], mybir.dt.uint16,
                        kind="ExternalOutput").ap()
    with tile.TileContext(nc) as tc:
        if loop:
            with tc.For_i(0, loop, 1):
                _mha_kernel(tc, x, wqk, wv, bqk, bv, wo, tri, yp)
        else:
            for _ in range(repeat):
                _mha_kernel(tc, x, wqk, wv, bqk, bv, wo, tri, yp)
    nc.compile()
    _CACHE[key] = nc
    return nc


def _shard_inputs(x, Wqkv, bqkv, Wo, bo, mask):
    x = np.asarray(x, dtype=np.float32)
    Wqkv = np.asarray(Wqkv, dtype=np.float32)
    bqkv = np.asarray(bqkv, dtype=np.float32)
    Wo = np.asarray(Wo, dtype=np.float32)
    mask = np.asarray(mask, dtype=np.float32)

    # causal 0/1 lower triangle [k, q] for a diagonal 128-block, duplicated
    blk = mask[0:128, 0:128]                     # [q, k] additive mask
    tri = (blk.T == 0.0).astype(np.float32)      # [k, q]
    tri2 = np.tile(tri, (1, 2))
    tri2_u16 = tri2.astype(ml_dtypes.bfloat16).view(np.uint16)

    in_maps = []
    for c in range(8):
        b, g = divmod(c, 4)
        h0 = NH * g
        qk_cols = []
        for t in range(2):
            for h in range(NH):
                base = 3 * HD * (h0 + h) + t * HD
                qk_cols.extend(range(base, base + HD))
        qk_cols = np.array(qk_cols)
        v_cols = []
        for h in range(NH):
            base = 3 * HD * (h0 + h) + 2 * HD
            v_cols.extend(range(base, base + HD))
        v_cols = np.array(v_cols)

        xt = np.ascontiguousarray(x[b].T).astype(ml_dtypes.bfloat16)
        wqk = np.ascontiguousarray(Wqkv[:, qk_cols]).astype(ml_dtypes.bfloat16)
        wv = np.ascontiguousarray(Wqkv[:, v_cols]).astype(ml_dtypes.bfloat16)
        bqk = bqkv[qk_cols].reshape(4, 128).T.copy()
        bv = bqkv[v_cols].reshape(1, 256).astype(ml_dtypes.bfloat16)
        wo = np.ascontiguousarray(
            Wo[HD * h0:HD * h0 + NH * HD, :]).astype(ml_dtypes.bfloat16)
        in_maps.append({
            "x": xt.view(np.uint16),
            "wqk": wqk.view(np.uint16),
            "wv": wv.view(np.uint16),
            "bqk": np.ascontiguousarray(bqk),
            "bv": bv.view(np.uint16),
            "wo": wo.view(np.uint16),
            "tri": tri2_u16,
        })
    return in_maps


def kernel(x, Wqkv, bqkv, Wo, bo, mask):
    global LAST_RESULT
    nc = _build()
    in_maps = _shard_inputs(x, Wqkv, bqkv, Wo, bo, mask)
    trace = bool(int(os.environ.get("KERNEL_TRACE", "0")))
    res = run_bass_kernel_spmd(nc, in_maps, list(range(8)), trace=trace)
    LAST_RESULT = res
    bo = np.asarray(bo, dtype=np.float32)
    y = np.empty((B, S, E), dtype=np.float32)
    for b in range(B):
        acc = np.zeros((S, E), dtype=np.float32)
        for g in range(4):
            part = res.results[4 * b + g]["yp"].view(ml_dtypes.bfloat16)
            acc = acc + part.astype(np.float32)
        y[b] = acc + bo[None, :]
    return y


# revision 15
# speedup vs baseline: 1.0935x; 1.0935x over previous
"""Multi-head attention (B=2, S=2048, E=1024, H=16, hd=64) on 8 trn2 cores.

Sharding: core c handles batch b = c//4 and 4 heads h0 = 4*(c%4).
Each core computes its heads' attention output projected through its rows
of Wo (tensor-parallel row split); the host sums the 4 partials per batch
and adds bo.

v2 design (vs v1 baseline at 241us):
  - bf16 q/k/v/exps/oT/Wo (fp8 fails the 2e-2 tolerance; bf16 adds ~1e-3)
  - v computed directly in [k, d] layout from the QKV matmul with xT as
    the stationary operand (no PE transposes, no extra DVE copies); v bias
    folded in via a K=1 ones-row matmul that opens the PSUM group
  - score PSUM tiles span 2 banks [128, 1024] covering both heads of a
    pair -> one exp activation per k-tile (halves ACT fixed overhead)
  - causal mask multiply narrowed to the 128-wide diagonal triangle
  - exp only over the live [q0:] region (stale PSUM cols would make
    Inf*0 = NaN)
  - softmax normalization: reciprocal of the den row (PE-accumulated via
    a ones column in v1), broadcast across 128 partitions with a [2,128]
    0/1 sel matmul, one per head-pair
  - software pipelining: QKV/V chunk closures for seq-block n+1 are
    popped into attention(n)'s ACT-bound inner loop so the PE never idles
  - y evacuation split between DVE and ACT; output DMA per 128-row block
"""

import os
import sys

sys.path.insert(0, "/opt/trn_rl_repo")

from contextlib import ExitStack

import ml_dtypes
import numpy as np

import concourse.bass as bass
import concourse.tile as tile
from concourse import bacc, mybir
from concourse._compat import with_exitstack
from concourse.bass_utils import run_bass_kernel_spmd

B, S, E, H = 2, 2048, 1024, 16
HD = E // H            # 64
NH = 4                 # heads per core
ET = E // 128          # 8 e-tiles
KT = S // 128          # 16 k tiles
VP = 66                # v1 per-head stride: 64 v cols + 1 ones + 1 pad
F32 = mybir.dt.float32
F32R = mybir.dt.float32r
BF16 = mybir.dt.bfloat16
EXP = mybir.ActivationFunctionType.Exp
COPY = mybir.ActivationFunctionType.Copy

_CACHE = {}
LAST_RESULT = None


@with_exitstack
def _mha_kernel(ctx: ExitStack, tc: tile.TileContext, x, wqk, wv, bqk, bv, wo,
                tri, yp):
    nc = tc.nc
    ctx.enter_context(nc.allow_low_precision(
        reason="bf16 intermediates verified against 2e-2 rel tolerance"))

    const = ctx.enter_context(tc.tile_pool(name="const", bufs=1))
    work = ctx.enter_context(tc.tile_pool(name="work", bufs=1))
    psum = ctx.enter_context(tc.tile_pool(name="psum", bufs=1, space="PSUM"))

    # ---- persistent SBUF tensors ----
    xT = const.tile([128, ET, S], BF16)          # x[b] transposed, bf16
    WQK = const.tile([128, ET, 512], BF16)       # q,k weight cols (4 m-tiles)
    WV = const.tile([128, ET, 256], BF16)        # v weight cols
    WOr = const.tile([128, 2, E], BF16)          # Wo rows, 2 hpt tiles
    BQK = const.tile([128, 4], F32)              # q,k bias per m-tile
    BV = const.tile([1, 256], BF16)              # v bias row
    ONE1 = const.tile([1, 128], BF16)            # ones row for bias matmul
    TRI = const.tile([128, 256], BF16)           # causal 0/1 triangle, x2
    qT = const.tile([128, 2, S], BF16)           # [d(2 heads), hp, s]
    kT = const.tile([128, 2, S], BF16)
    v1 = const.tile([128, KT, NH * VP], BF16)    # [k, kt, h*VP + d | ones]

    # ---- constant loads: merged strided transfers, split over both rings
    nc.sync.dma_start(WQK[:, :, :],
                      wqk.rearrange("(t p) c -> p t c", p=128).bitcast(BF16))
    nc.scalar.dma_start(WV[:, :, :],
                        wv.rearrange("(t p) c -> p t c", p=128).bitcast(BF16))
    nc.scalar.dma_start(BQK[:], bqk[:, :])
    nc.scalar.dma_start(BV[:], bv[:, :].bitcast(BF16))
    for t in range(ET):
        eng = nc.sync if t % 2 == 0 else nc.scalar
        eng.dma_start(xT[:, t, :], x[t * 128:(t + 1) * 128, :].bitcast(BF16))
    nc.scalar.dma_start(TRI[:], tri[:, :].bitcast(BF16))
    nc.scalar.dma_start(WOr[:, :, :],
                        wo.rearrange("(t p) c -> p t c", p=128).bitcast(BF16))
    nc.vector.memset(ONE1[:], 1.0)
    for h in range(NH):
        nc.vector.memset(v1[:, :, h * VP + HD:h * VP + HD + 1], 1.0)

    # ---- QKV producer chunks for seq block n (each ~0.9us of PE work) ----
    def qkv_chunks(n):
        ncol = slice(n * 512, (n + 1) * 512)
        chunks = []
        for m in range(4):
            box = {}

            def c0(m=m, box=box):
                ps = psum.tile([128, 512], F32, name=f"qk{m}", tag="mm", bufs=2)
                box["ps"] = ps
                for et in range(4):
                    nc.tensor.matmul(ps[:], WQK[:, et, m * 128:(m + 1) * 128],
                                     xT[:, et, ncol], start=(et == 0), stop=False)

            def c1(m=m, box=box):
                ps = box["ps"]
                for et in range(4, 8):
                    nc.tensor.matmul(ps[:], WQK[:, et, m * 128:(m + 1) * 128],
                                     xT[:, et, ncol], start=False, stop=(et == 7))
                typ, hp = m // 2, m % 2
                dst = (qT if typ == 0 else kT)[:, hp, ncol]
                nc.vector.tensor_scalar_add(dst, ps[:], BQK[:, m:m + 1])

            chunks += [c0, c1]
        for st in range(4 * n, 4 * n + 4):
            box = {}
            scol = slice(st * 128, (st + 1) * 128)

            def v0(scol=scol, st=st, box=box):
                vps = psum.tile([128, 256], F32, name=f"v{st}", tag="mm", bufs=2)
                box["ps"] = vps
                nc.tensor.matmul(vps[:], ONE1[:], BV[:], start=True, stop=False)
                for et in range(4):
                    nc.tensor.matmul(vps[:], xT[:, et, scol], WV[:, et, :],
                                     start=False, stop=False)

            def v1c(scol=scol, st=st, box=box):
                vps = box["ps"]
                for et in range(4, 8):
                    nc.tensor.matmul(vps[:], xT[:, et, scol], WV[:, et, :],
                                     start=False, stop=(et == 7))
                nc.vector.tensor_copy(
                    v1[:, st, :].rearrange("p (h c) -> p h c", h=NH)[:, :, :HD],
                    vps[:].rearrange("p (h c) -> p h c", h=NH))

            chunks += [v0, v1c]
        return chunks

    def pop(queue):
        if queue:
            queue.pop(0)()

    # ---- attention for query super-block qsb ----
    def attention(qsb, queue):
        nkt = 4 * (qsb + 1)
        qcol0 = qsb * 512
        oT = work.tile([128, 2, 512], BF16, name="oT", tag="oT", bufs=2)
        for hp in range(2):
            ops = [
                psum.tile([65, 512], F32, name=f"ops{h2}", tag=f"ops{h2}", bufs=1)
                for h2 in range(2)
            ]

            def emit_av(kt, q0, exv):
                for h2 in range(2):
                    h = 2 * hp + h2
                    nc.tensor.matmul(
                        ops[h2][:, q0:],
                        v1[:, kt, h * VP:h * VP + HD + 1],
                        exv[:, h2, q0:],
                        start=(kt == 0),
                        stop=(kt == nkt - 1),
                    )

            pend = None
            for kt in range(nkt):
                j = kt - 4 * qsb
                q0 = max(0, j * 128)
                kcol = slice(kt * 128, (kt + 1) * 128)
                sc = psum.tile([128, 1024], F32, name="sc", tag="sc", bufs=2)
                for h2 in range(2):
                    b0 = 64 * h2
                    nc.tensor.matmul(
                        sc[:, h2 * 512 + q0:(h2 + 1) * 512],
                        kT[b0:b0 + 64, hp, kcol],
                        qT[b0:b0 + 64, hp, qcol0 + q0:qcol0 + 512],
                        start=True, stop=True)
                ex = work.tile([128, 1024], BF16, name="ex", tag="ex", bufs=6)
                exv = ex[:].rearrange("p (h q) -> p h q", h=2)
                scv = sc[:].rearrange("p (h q) -> p h q", h=2)
                nc.scalar.activation(exv[:, :, q0:], scv[:, :, q0:], EXP,
                                     scale=0.125)
                if j >= 0:
                    nc.gpsimd.tensor_mul(
                        exv[:, :, q0:q0 + 128], exv[:, :, q0:q0 + 128],
                        TRI[:].rearrange("p (h q) -> p h q", h=2))
                if pend is not None:
                    emit_av(*pend)
                else:
                    pop(queue)
                pend = (kt, q0, exv)
                pop(queue)
            emit_av(*pend)

            # normalization: oT[d, q] = ops[d, q] * (1 / den[q])
            for h2 in range(2):
                b0 = 64 * h2
                rc = work.tile([1, 512], BF16, name="rc", tag="rc", bufs=4)
                nc.vector.reciprocal(rc[:], ops[h2][64:65, :])
                rb = psum.tile([64, 512], F32, name="rb", tag="mm", bufs=2)
                nc.tensor.matmul(rb[:], ONE1[:, 0:64], rc[:], start=True,
                                 stop=True)
                rbs = work.tile([64, 512], BF16, name="rbs", tag="rbs", bufs=2)
                nc.vector.tensor_copy(rbs[:], rb[:])
                pop(queue)
                nc.vector.tensor_mul(oT[b0:b0 + 64, hp, :], ops[h2][0:64, :],
                                     rbs[:])
        return oT

    # ---- output projection chunks for query super-block qsb ----
    def wo_chunks(qsb, oT):
        def mk(qb2):
            def c(qb2=qb2):
                qb = qsb * 4 + qb2
                yps = [
                    psum.tile([128, 512], F32, name=f"yps{ec}", tag="mm",
                              bufs=2)
                    for ec in range(2)
                ]
                for hpt in range(2):
                    for ec in range(2):
                        nc.tensor.matmul(
                            yps[ec][:], oT[:, hpt, qb2 * 128:(qb2 + 1) * 128],
                            WOr[:, hpt, ec * 512:(ec + 1) * 512],
                            start=(hpt == 0), stop=(hpt == 1))
                yt = work.tile([128, E], BF16, tag="yt", bufs=3)
                nc.vector.tensor_copy(yt[:, 0:512], yps[0][:])
                nc.vector.tensor_copy(yt[:, 512:1024], yps[1][:])
                nc.sync.dma_start(yp[qb * 128:(qb + 1) * 128, :],
                              yt[:].bitcast(mybir.dt.uint16))
            return c
        return [mk(qb2) for qb2 in range(4)]

    # ---- main pipeline ----
    for c in qkv_chunks(0):
        c()
    queue = []
    for n in range(4):
        if n < 3:
            queue += qkv_chunks(n + 1)
        oT = attention(n, queue)
        for c in wo_chunks(n, oT):
            c()
            pop(queue)
    for c in queue:
        c()


def _build(repeat=1, loop=0):
    key = ("nc", repeat, loop)
    if key in _CACHE:
        return _CACHE[key]
    nc = bacc.Bacc("TRN2", target_bir_lowering=False, debug=False, num_devices=8)
    x = nc.dram_tensor("x", [E, S], mybir.dt.uint16, kind="ExternalInput").ap()
    wqk = nc.dram_tensor("wqk", [E, 512], mybir.dt.uint16,
                         kind="ExternalInput").ap()
    wv = nc.dram_tensor("wv", [E, 256], mybir.dt.uint16,
                        kind="ExternalInput").ap()
    bqk = nc.dram_tensor("bqk", [128, 4], F32, kind="ExternalInput").ap()
    bv = nc.dram_tensor("bv", [1, 256], mybir.dt.uint16,
                        kind="ExternalInput").ap()
    wo = nc.dram_tensor("wo", [NH * HD, E], mybir.dt.uint16,
                        kind="ExternalInput").ap()
    tri = nc.dram_tensor("tri", [128, 256], mybir.dt.uint16,
                         kind="ExternalInput").ap()
    yp = nc.dram_tensor("yp", [S, E], mybir.dt.uint16,
                        kind="ExternalOutput").ap()
    with tile.TileContext(nc) as tc:
        if loop:
            with tc.For_i(0, loop, 1):
                _mha_kernel(tc, x, wqk, wv, bqk, bv, wo, tri, yp)
        else:
            for _ in range(repeat):
                _mha_kernel(tc, x, wqk, wv, bqk, bv, wo, tri, yp)
    nc.compile()
    _CACHE[key] = nc
    return nc


def _shard_inputs(x, Wqkv, bqkv, Wo, bo, mask):
    x = np.asarray(x, dtype=np.float32)
    Wqkv = np.asarray(Wqkv, dtype=np.float32)
    bqkv = np.asarray(bqkv, dtype=np.float32)
    Wo = np.asarray(Wo, dtype=np.float32)
    mask = np.asarray(mask, dtype=np.float32)

    # causal 0/1 lower triangle [k, q] for a diagonal 128-block, duplicated
    blk = mask[0:128, 0:128]                     # [q, k] additive mask
    tri = (blk.T == 0.0).astype(np.float32)      # [k, q]
    tri2 = np.tile(tri, (1, 2))
    tri2_u16 = tri2.astype(ml_dtypes.bfloat16).view(np.uint16)

    in_maps = []
    for c in range(8):
        b, g = divmod(c, 4)
        h0 = NH * g
        qk_cols = []
        for t in range(2):
            for h in range(NH):
                base = 3 * HD * (h0 + h) + t * HD
                qk_cols.extend(range(base, base + HD))
        qk_cols = np.array(qk_cols)
        v_cols = []
        for h in range(NH):
            base = 3 * HD * (h0 + h) + 2 * HD
            v_cols.extend(range(base, base + HD))
        v_cols = np.array(v_cols)

        xt = np.ascontiguousarray(x[b].T).astype(ml_dtypes.bfloat16)
        wqk = np.ascontiguousarray(Wqkv[:, qk_cols]).astype(ml_dtypes.bfloat16)
        wv = np.ascontiguousarray(Wqkv[:, v_cols]).astype(ml_dtypes.bfloat16)
        bqk = bqkv[qk_cols].reshape(4, 128).T.copy()
        bv = bqkv[v_cols].reshape(1, 256).astype(ml_dtypes.bfloat16)
        wo = np.ascontiguousarray(
            Wo[HD * h0:HD * h0 + NH * HD, :]).astype(ml_dtypes.bfloat16)
        in_maps.append({
            "x": xt.view(np.uint16),
            "wqk": wqk.view(np.uint16),
            "wv": wv.view(np.uint16),
            "bqk": np.ascontiguousarray(bqk),
            "bv": bv.view(np.uint16),
            "wo": wo.view(np.uint16),
            "tri": tri2_u16,
        })
    return in_maps


def kernel(x, Wqkv, bqkv, Wo, bo, mask):
    global LAST_RESULT
    nc = _build()
    in_maps = _shard_inputs(x, Wqkv, bqkv, Wo, bo, mask)
    trace = bool(int(os.environ.get("KERNEL_TRACE", "0")))
    res = run_bass_kernel_spmd(nc, in_maps, list(range(8)), trace=trace)
    LAST_RESULT = res
    bo = np.asarray(bo, dtype=np.float32)
    y = np.empty((B, S, E), dtype=np.float32)
    for b in range(B):
        acc = np.zeros((S, E), dtype=np.float32)
        for g in range(4):
            part = res.results[4 * b + g]["yp"].view(ml_dtypes.bfloat16)
            acc = acc + part.astype(np.float32)
        y[b] = acc + bo[None, :]
    return y


# revision 17
# speedup vs baseline: 1.1943x; 1.0922x over previous
"""Multi-head attention (B=2, S=2048, E=1024, H=16, hd=64) on 8 trn2 cores.

Sharding: core c handles batch b = c//4 and 4 heads h0 = 4*(c%4).
Each core computes its heads' attention output projected through its rows
of Wo (tensor-parallel row split); the host sums the 4 partials per batch
and adds bo.

v2 design (vs v1 baseline at 241us):
  - bf16 q/k/v/exps/oT/Wo (fp8 fails the 2e-2 tolerance; bf16 adds ~1e-3)
  - v computed directly in [k, d] layout from the QKV matmul with xT as
    the stationary operand (no PE transposes, no extra DVE copies); v bias
    folded in via a K=1 ones-row matmul that opens the PSUM group
  - score PSUM tiles span 2 banks [128, 1024] covering both heads of a
    pair -> one exp activation per k-tile (halves ACT fixed overhead)
  - causal mask multiply narrowed to the 128-wide diagonal triangle
  - exp only over the live [q0:] region (stale PSUM cols would make
    Inf*0 = NaN)
  - softmax normalization: reciprocal of the den row (PE-accumulated via
    a ones column in v1), broadcast across 128 partitions with a [2,128]
    0/1 sel matmul, one per head-pair
  - software pipelining: QKV/V chunk closures for seq-block n+1 are
    popped into attention(n)'s ACT-bound inner loop so the PE never idles
  - y evacuation split between DVE and ACT; output DMA per 128-row block
"""

import os
import sys

sys.path.insert(0, "/opt/trn_rl_repo")

from contextlib import ExitStack

import ml_dtypes
import numpy as np

import concourse.bass as bass
import concourse.tile as tile
from concourse import bacc, mybir
from concourse._compat import with_exitstack
from concourse.bass_utils import run_bass_kernel_spmd

B, S, E, H = 2, 2048, 1024, 16
HD = E // H            # 64
NH = 4                 # heads per core
ET = E // 128          # 8 e-tiles
KT = S // 128          # 16 k tiles
VP = 66                # v1 per-head stride: 64 v cols + 1 ones + 1 pad
F32 = mybir.dt.float32
F32R = mybir.dt.float32r
BF16 = mybir.dt.bfloat16
EXP = mybir.ActivationFunctionType.Exp
COPY = mybir.ActivationFunctionType.Copy

_CACHE = {}
LAST_RESULT = None


@with_exitstack
def _mha_kernel(ctx: ExitStack, tc: tile.TileContext, x, wqk, wv, bqk, bv, wo,
                tri, yp):
    nc = tc.nc
    ctx.enter_context(nc.allow_low_precision(
        reason="bf16 intermediates verified against 2e-2 rel tolerance"))

    const = ctx.enter_context(tc.tile_pool(name="const", bufs=1))
    work = ctx.enter_context(tc.tile_pool(name="work", bufs=1))
    psum = ctx.enter_context(tc.tile_pool(name="psum", bufs=1, space="PSUM"))

    # ---- persistent SBUF tensors ----
    xT = const.tile([128, ET, S], BF16)          # x[b] transposed, bf16
    WQK = const.tile([128, ET, 512], BF16)       # q,k weight cols (4 m-tiles)
    WV = const.tile([128, ET, 256], BF16)        # v weight cols
    WOr = const.tile([128, 2, E], BF16)          # Wo rows, 2 hpt tiles
    BQK = const.tile([128, 4], F32)              # q,k bias per m-tile
    BV = const.tile([1, 256], BF16)              # v bias row
    ONE1 = const.tile([1, 128], BF16)            # ones row for bias matmul
    TRI = const.tile([128, 256], BF16)           # causal 0/1 triangle, x2
    qT = const.tile([128, 2, S], BF16)           # [d(2 heads), hp, s]
    kT = const.tile([128, 2, S], BF16)
    v1 = const.tile([128, KT, NH * VP], BF16)    # [k, kt, h*VP + d | ones]

    # ---- constant loads: merged strided transfers, split over both rings
    nc.sync.dma_start(WQK[:, :, :],
                      wqk.rearrange("(t p) c -> p t c", p=128).bitcast(BF16))
    nc.scalar.dma_start(WV[:, :, :],
                        wv.rearrange("(t p) c -> p t c", p=128).bitcast(BF16))
    nc.scalar.dma_start(BQK[:], bqk[:, :])
    nc.scalar.dma_start(BV[:], bv[:, :].bitcast(BF16))
    for t in range(ET):
        eng = nc.sync if t % 2 == 0 else nc.scalar
        eng.dma_start(xT[:, t, :], x[t * 128:(t + 1) * 128, :].bitcast(BF16))
    nc.scalar.dma_start(TRI[:], tri[:, :].bitcast(BF16))
    nc.scalar.dma_start(WOr[:, :, :],
                        wo.rearrange("(t p) c -> p t c", p=128).bitcast(BF16))
    nc.vector.memset(ONE1[:], 1.0)
    for h in range(NH):
        nc.vector.memset(v1[:, :, h * VP + HD:h * VP + HD + 1], 1.0)

    # ---- QKV producer chunks for seq block n (each ~0.9us of PE work) ----
    def qkv_chunks(n):
        ncol = slice(n * 512, (n + 1) * 512)
        chunks = []
        for m in range(4):
            box = {}

            def c0(m=m, box=box):
                ps = psum.tile([128, 512], F32, name=f"qk{m}", tag="mm", bufs=2)
                box["ps"] = ps
                for et in range(4):
                    nc.tensor.matmul(ps[:], WQK[:, et, m * 128:(m + 1) * 128],
                                     xT[:, et, ncol], start=(et == 0), stop=False)

            def c1(m=m, box=box):
                ps = box["ps"]
                for et in range(4, 8):
                    nc.tensor.matmul(ps[:], WQK[:, et, m * 128:(m + 1) * 128],
                                     xT[:, et, ncol], start=False, stop=(et == 7))
                typ, hp = m // 2, m % 2
                dst = (qT if typ == 0 else kT)[:, hp, ncol]
                nc.vector.tensor_scalar_add(dst, ps[:], BQK[:, m:m + 1])

            chunks += [c0, c1]
        for st in range(4 * n, 4 * n + 4):
            box = {}
            scol = slice(st * 128, (st + 1) * 128)

            def v0(scol=scol, st=st, box=box):
                vps = psum.tile([128, 256], F32, name=f"v{st}", tag="mm", bufs=2)
                box["ps"] = vps
                nc.tensor.matmul(vps[:], ONE1[:], BV[:], start=True, stop=False)
                for et in range(4):
                    nc.tensor.matmul(vps[:], xT[:, et, scol], WV[:, et, :],
                                     start=False, stop=False)

            def v1c(scol=scol, st=st, box=box):
                vps = box["ps"]
                for et in range(4, 8):
                    nc.tensor.matmul(vps[:], xT[:, et, scol], WV[:, et, :],
                                     start=False, stop=(et == 7))
                nc.vector.tensor_copy(
                    v1[:, st, :].rearrange("p (h c) -> p h c", h=NH)[:, :, :HD],
                    vps[:].rearrange("p (h c) -> p h c", h=NH))

            chunks += [v0, v1c]
        return chunks

    def pop(queue):
        if queue:
            queue.pop(0)()

    # ---- attention for query super-block qsb ----
    def attention(qsb, queue):
        nkt = 4 * (qsb + 1)
        qcol0 = qsb * 512
        oT = work.tile([128, 2, 512], BF16, name="oT", tag="oT", bufs=2)
        for hp in range(2):
            ops = [
                psum.tile([65, 512], F32, name=f"ops{h2}", tag=f"ops{h2}", bufs=1)
                for h2 in range(2)
            ]

            def emit_av(kt, q0, exv):
                for h2 in range(2):
                    h = 2 * hp + h2
                    nc.tensor.matmul(
                        ops[h2][:, q0:],
                        v1[:, kt, h * VP:h * VP + HD + 1],
                        exv[:, h2, q0:],
                        start=(kt == 0),
                        stop=(kt == nkt - 1),
                    )

            pend = None
            for kt in range(nkt):
                j = kt - 4 * qsb
                q0 = max(0, j * 128)
                kcol = slice(kt * 128, (kt + 1) * 128)
                sc = psum.tile([128, 1024], F32, name="sc", tag="sc", bufs=2)
                for h2 in range(2):
                    b0 = 64 * h2
                    nc.tensor.matmul(
                        sc[:, h2 * 512 + q0:(h2 + 1) * 512],
                        kT[b0:b0 + 64, hp, kcol],
                        qT[b0:b0 + 64, hp, qcol0 + q0:qcol0 + 512],
                        start=True, stop=True)
                ex = work.tile([128, 1024], BF16, name="ex", tag="ex", bufs=6)
                exv = ex[:].rearrange("p (h q) -> p h q", h=2)
                scv = sc[:].rearrange("p (h q) -> p h q", h=2)
                nc.scalar.activation(exv[:, :, q0:], scv[:, :, q0:], EXP,
                                     scale=0.125)
                if j >= 0:
                    nc.gpsimd.tensor_mul(
                        exv[:, :, q0:q0 + 128], exv[:, :, q0:q0 + 128],
                        TRI[:].rearrange("p (h q) -> p h q", h=2))
                if pend is not None:
                    emit_av(*pend)
                else:
                    pop(queue)
                pend = (kt, q0, exv)
                pop(queue)
            emit_av(*pend)

            # normalization: oT[d, q] = ops[d, q] * (1 / den[q])
            for h2 in range(2):
                b0 = 64 * h2
                rc = work.tile([1, 512], BF16, name="rc", tag="rc", bufs=4)
                nc.vector.reciprocal(rc[:], ops[h2][64:65, :])
                rb = psum.tile([64, 512], F32, name="rb", tag="mm", bufs=2)
                nc.tensor.matmul(rb[:], ONE1[:, 0:64], rc[:], start=True,
                                 stop=True)
                rbs = work.tile([64, 512], BF16, name="rbs", tag="rbs", bufs=2)
                nc.vector.tensor_copy(rbs[:], rb[:])
                pop(queue)
                nc.vector.tensor_mul(oT[b0:b0 + 64, hp, :], ops[h2][0:64, :],
                                     rbs[:])
        return oT

    # ---- output projection chunks for query super-block qsb ----
    def wo_chunks(qsb, oT):
        def mk(qb2):
            def c(qb2=qb2):
                qb = qsb * 4 + qb2
                yps = [
                    psum.tile([128, 512], F32, name=f"yps{ec}", tag="mm",
                              bufs=2)
                    for ec in range(2)
                ]
                for hpt in range(2):
                    for ec in range(2):
                        nc.tensor.matmul(
                            yps[ec][:], oT[:, hpt, qb2 * 128:(qb2 + 1) * 128],
                            WOr[:, hpt, ec * 512:(ec + 1) * 512],
                            start=(hpt == 0), stop=(hpt == 1))
                yt = work.tile([128, E], BF16, tag="yt", bufs=3)
                nc.vector.tensor_copy(yt[:, 0:512], yps[0][:])
                nc.vector.tensor_copy(yt[:, 512:1024], yps[1][:])
                nc.sync.dma_start(yp[qb * 128:(qb + 1) * 128, :],
                              yt[:].bitcast(mybir.dt.uint16))
            return c
        return [mk(qb2) for qb2 in range(4)]

    # ---- main pipeline ----
    for c in qkv_chunks(0):
        c()
    queue = []
    for n in range(4):
        if n < 3:
            queue += qkv_chunks(n + 1)
        oT = attention(n, queue)
        for c in wo_chunks(n, oT):
            c()
            pop(queue)
    for c in queue:
        c()


def _build(repeat=1, loop=0):
    key = ("nc", repeat, loop)
    if key in _CACHE:
        return _CACHE[key]
    nc = bacc.Bacc("TRN2", target_bir_lowering=False, debug=False, num_devices=8)
    x = nc.dram_tensor("x", [E, S], mybir.dt.uint16, kind="ExternalInput").ap()
    wqk = nc.dram_tensor("wqk", [E, 512], mybir.dt.uint16,
                         kind="ExternalInput").ap()
    wv = nc.dram_tensor("wv", [E, 256], mybir.dt.uint16,
                        kind="ExternalInput").ap()
    bqk = nc.dram_tensor("bqk", [128, 4], F32, kind="ExternalInput").ap()
    bv = nc.dram_tensor("bv", [1, 256], mybir.dt.uint16,
                        kind="ExternalInput").ap()
    wo = nc.dram_tensor("wo", [NH * HD, E], mybir.dt.uint16,
                        kind="ExternalInput").ap()
    tri = nc.dram_tensor("tri", [128, 256], mybir.dt.uint16,
                         kind="ExternalInput").ap()
    yp = nc.dram_tensor("yp", [S, E], mybir.dt.uint16,
                        kind="ExternalOutput").ap()
    with tile.TileContext(nc) as tc:
        if loop:
            with tc.For_i(0, loop, 1):
                _mha_kernel(tc, x, wqk, wv, bqk, bv, wo, tri, yp)
        else:
            for _ in range(repeat):
                _mha_kernel(tc, x, wqk, wv, bqk, bv, wo, tri, yp)
    nc.compile()
    _CACHE[key] = nc
    return nc


def _shard_inputs(x, Wqkv, bqkv, Wo, bo, mask):
    x = np.asarray(x, dtype=np.float32)
    Wqkv = np.asarray(Wqkv, dtype=np.float32)
    bqkv = np.asarray(bqkv, dtype=np.float32)
    Wo = np.asarray(Wo, dtype=np.float32)
    mask = np.asarray(mask, dtype=np.float32)

    # causal 0/1 lower triangle [k, q] for a diagonal 128-block, duplicated
    blk = mask[0:128, 0:128]                     # [q, k] additive mask
    tri = (blk.T == 0.0).astype(np.float32)      # [k, q]
    tri2 = np.tile(tri, (1, 2))
    tri2_u16 = tri2.astype(ml_dtypes.bfloat16).view(np.uint16)

    in_maps = []
    for c in range(8):
        b, g = divmod(c, 4)
        h0 = NH * g
        qk_cols = []
        for t in range(2):
            for h in range(NH):
                base = 3 * HD * (h0 + h) + t * HD
                qk_cols.extend(range(base, base + HD))
        qk_cols = np.array(qk_cols)
        v_cols = []
        for h in range(NH):
            base = 3 * HD * (h0 + h) + 2 * HD
            v_cols.extend(range(base, base + HD))
        v_cols = np.array(v_cols)

        xt = np.ascontiguousarray(x[b].T).astype(ml_dtypes.bfloat16)
        wqk = np.ascontiguousarray(Wqkv[:, qk_cols]).astype(ml_dtypes.bfloat16)
        wv = np.ascontiguousarray(Wqkv[:, v_cols]).astype(ml_dtypes.bfloat16)
        bqk = bqkv[qk_cols].reshape(4, 128).T.copy()
        bv = bqkv[v_cols].reshape(1, 256).astype(ml_dtypes.bfloat16)
        wo = np.ascontiguousarray(
            Wo[HD * h0:HD * h0 + NH * HD, :]).astype(ml_dtypes.bfloat16)
        in_maps.append({
            "x": xt.view(np.uint16),
            "wqk": wqk.view(np.uint16),
            "wv": wv.view(np.uint16),
            "bqk": np.ascontiguousarray(bqk),
            "bv": bv.view(np.uint16),
            "wo": wo.view(np.uint16),
            "tri": tri2_u16,
        })
    return in_maps


def kernel(x, Wqkv, bqkv, Wo, bo, mask):
    global LAST_RESULT
    nc = _build()
    in_maps = _shard_inputs(x, Wqkv, bqkv, Wo, bo, mask)
    trace = bool(int(os.environ.get("KERNEL_TRACE", "0")))
    res = run_bass_kernel_spmd(nc, in_maps, list(range(8)), trace=trace)
    LAST_RESULT = res
    bo = np.asarray(bo, dtype=np.float32)
    y = np.empty((B, S, E), dtype=np.float32)
    for b in range(B):
        acc = np.zeros((S, E), dtype=np.float32)
        for g in range(4):
            part = res.results[4 * b + g]["yp"].view(ml_dtypes.bfloat16)
            acc = acc + part.astype(np.float32)
        y[b] = acc + bo[None, :]
    return y
